# revision 1
# baseline (speedup 1.0000x reference)
# Trainium2 Bass kernel for the 4-branch cross-attention block.
#
# Problem: N=4 batches, L1=L2=1024, D=512, H=8 heads of 64.
#   q1,k1,v1 = proj(input1); q2,k2,v2 = proj(input2)
#   four attention branches (q1k1v1, q1k2v2, q2k1v1, q2k2v2), masked softmax
#   over the key axis, outputs averaged pairwise.
#
# Sharding: 8 cores = 4 batches x 2 head-groups (4 heads each). SPMD — one
# program, per-core data.
#
# Device-side layout trick: attention runs entirely in the transposed
# "ST" layout (keys on partitions, queries on the free axis):
#   ST   = K @ Q^T            (lhsT = kT slice, rhs = qT)
#   P    = exp(ST + bias_k)   (ACT; bias is the -10000 column mask, per
#                              partition; no max subtraction — logits are
#                              bounded so fp32 exp cannot overflow)
#   O^T  = [V | 1]^T @ P      (ones column yields the softmax denominators
#                              in psum row 64)
#   out  = (O^T[0:64] * bcast(0.5 * mask_q / s)) accumulated over branches
# Projections/QK in fp16 (1 cyc/row, ~5e-4 rel err), P*V in bf16 (P can
# reach e^44 which overflows fp16), rank-1 broadcasts in fp32r.
# QK streams the full 128-partition qT via zero-padded kT blocks.
# Host does all pure transposes (inputs, weights, outputs).

import sys

sys.path.insert(0, "/opt/trn_rl_repo")

import numpy as np

import concourse.bacc as bacc
import concourse.mybir as mybir
import concourse.tile as tile
from concourse.bass_utils import run_bass_kernel_spmd

F32 = mybir.dt.float32
F32R = mybir.dt.float32r
F16 = mybir.dt.float16
BF16 = mybir.dt.bfloat16
EXP = mybir.ActivationFunctionType.Exp

L = 1024  # sequence length (both sides)
D = 512  # hidden
NB = 4  # batches
HPG = 4  # heads per core (head group)
HD = 64  # head size
OG = HPG * HD  # output channels per core = 256
KT = L // 128  # 8 key tiles
DT = D // 128  # 4 contraction tiles for projections
INF = 10000.0

_NC = None  # cached compiled program
TRACE = False  # set by test harness to capture an NTFF profile
LAST_RESULT = None  # full BassKernelResults of the last run (for profiling)


def _tt(pool, shape, dtype, tag):
    return pool.tile(shape, dtype, tag=tag, name=tag)


def _install_ntff_hook():
    # antenv.axon_hooks is absent in this image; provide it so
    # run_bass_kernel_spmd(trace=True) can capture NTFF profiles.
    import types, contextlib, ctypes

    if "antenv.axon_hooks" in sys.modules:
        return
    lib = ctypes.CDLL("/opt/axon/libaxon_pjrt.so")
    lib.axon_start_nrt_profile.argtypes = [
        ctypes.POINTER(ctypes.c_int64),
        ctypes.c_size_t,
    ]
    lib.axon_start_nrt_profile.restype = ctypes.c_int64
    lib.axon_stop_nrt_profile.argtypes = [ctypes.c_char_p]
    lib.axon_stop_nrt_profile.restype = ctypes.c_int64

    @contextlib.contextmanager
    def _hook(output_dir, device_ids):
        import jax

        jax.devices()
        if device_ids:
            ids = (ctypes.c_int64 * len(device_ids))(*device_ids)
            rc = lib.axon_start_nrt_profile(ids, len(device_ids))
        else:
            rc = lib.axon_start_nrt_profile(None, 0)
        if rc != 0:
            raise RuntimeError(f"axon_start_nrt_profile rc={rc}")
        try:
            yield
        finally:
            n = lib.axon_stop_nrt_profile(str(output_dir).encode())
            print(f"ntff profile: {n} file(s) in {output_dir}", file=sys.stderr)

    mod = types.ModuleType("antenv.axon_hooks")
    mod.get_axon_ntff_profile_hook = lambda: _hook
    mod.set_axon_ntff_profile_hook = lambda h: None
    sys.modules["antenv.axon_hooks"] = mod



def _build():
    nc = bacc.Bacc("TRN2", target_bir_lowering=False, debug=False, num_devices=8)

    x1T = nc.declare_dram_parameter("x1T", [D, L], F32, isOutput=False)
    x2T = nc.declare_dram_parameter("x2T", [D, L], F32, isOutput=False)
    ws = {}
    for wn in ("wq1", "wk1", "wv1", "wq2", "wk2", "wv2"):
        ws[wn] = nc.declare_dram_parameter(wn, [D, OG], F32, isOutput=False)
    bias1 = nc.declare_dram_parameter("bias1", [128, KT], F32, isOutput=False)
    bias2 = nc.declare_dram_parameter("bias2", [128, KT], F32, isOutput=False)
    hm1 = nc.declare_dram_parameter("hm1", [1, L], F32, isOutput=False)
    hm2 = nc.declare_dram_parameter("hm2", [1, L], F32, isOutput=False)
    out1T = nc.declare_dram_parameter("out1T", [OG, L], F32, isOutput=True)
    out2T = nc.declare_dram_parameter("out2T", [OG, L], F32, isOutput=True)

    with tile.TileContext(nc) as tc:
        with (
            tc.tile_pool(name="pers", bufs=1) as pers,
            tc.tile_pool(name="io", bufs=3) as io,
            tc.tile_pool(name="pt", bufs=3) as ptp,
            tc.tile_pool(name="sm", bufs=2) as smp,
            tc.tile_pool(name="st", bufs=2, space="PSUM") as stp,
            tc.tile_pool(name="acc", bufs=2, space="PSUM") as accp,
        ):
            # ---- load + round inputs to f32r ----
            def load_round(dram, ncols, ntiles, name, dt=F16):
                tiles = []
                for i in range(ntiles):
                    raw = _tt(io, [128, ncols], F32, f"raw{ncols}")
                    nc.sync.dma_start(raw[:], dram[i * 128 : (i + 1) * 128, :])
                    r = _tt(pers, [128, ncols], dt, f"{name}_{i}")
                    nc.vector.tensor_copy(r[:], raw[:])
                    tiles.append(r)
                return tiles

            x_r = {1: load_round(x1T, L, DT, "x1"), 2: load_round(x2T, L, DT, "x2")}
            w_r = {wn: load_round(ws[wn], ws[wn].shape[1], DT, wn) for wn in ws}

            # ---- small constants ----
            b_sb = {}
            for qs, dram in ((1, bias1), (2, bias2)):
                b = _tt(pers, [128, KT], F32, f"bias{qs}")
                nc.sync.dma_start(b[:], dram[:])
                b_sb[qs] = b
            hm_sb = {}
            for qs, dram in ((1, hm1), (2, hm2)):
                h = _tt(pers, [1, L], F32, f"hm{qs}")
                nc.sync.dma_start(h[:], dram[:])
                hm_sb[qs] = h
            ones64f = _tt(pers, [1, 64], F32, "ones64f")
            nc.vector.memset(ones64f[:], 1.0)
            ones64r = _tt(pers, [1, 64], F32R, "ones64r")
            nc.vector.tensor_copy(ones64r[:], ones64f[:])
            onescol = _tt(pers, [128, HPG, 1], F32, "onescol")
            nc.vector.memset(onescol[:], 1.0)

            # ---- projections ----
            # qT/kT layout [og, L] (2 tiles of [128, L]): out[o, l] += W^T x^T
            qkT = {}
            for name in ("q1", "k1", "q2", "k2"):
                side = int(name[1])
                w = w_r["w" + name]
                tiles = []
                for ot in range(2):
                    ps = _tt(stp, [128, L], F32, "st")
                    for dk in range(DT):
                        for nh in range(2):
                            nc.tensor.matmul(
                                ps[:, nh * 512 : (nh + 1) * 512],
                                w[dk][:, ot * 128 : (ot + 1) * 128],
                                x_r[side][dk][:, nh * 512 : (nh + 1) * 512],
                                start=(dk == 0),
                                stop=(dk == DT - 1),
                            )
                    t = _tt(pers, [128, L], F16, f"{name}T_{ot}")
                    nc.vector.tensor_copy(t[:], ps[:])
                    tiles.append(t)
                qkT[name] = tiles

            # zero-padded kT: lhsT [128, 128] per (head, kt) with the other
            # head's partition rows zeroed, so the QK rhs can stream the full
            # 128-partition qT tile at full SBUF bandwidth (a 64-partition
            # moving operand reads at half rate).
            kz = {}
            for ks in (1, 2):
                z = _tt(pers, [128, HPG * KT * 128], F16, f"kz{ks}")
                nc.vector.memset(z[:], 0.0)
                for h in range(HPG):
                    po = (h % 2) * 64
                    ht = h // 2
                    for kt in range(KT):
                        blk = h * KT + kt
                        nc.vector.tensor_copy(
                            z[po : po + HD, blk * 128 : (blk + 1) * 128],
                            qkT[f"k{ks}"][ht][po : po + HD, kt * 128 : (kt + 1) * 128],
                        )
                kz[ks] = z

            # v in natural layout with ones column: [128, HPG, 65] per l-tile
            v_e = {}
            for side in (1, 2):
                w = w_r[f"wv{side}"]
                tiles = []
                for lt in range(KT):
                    ps = _tt(stp, [128, OG], F32, "st")
                    for dk in range(DT):
                        nc.tensor.matmul(
                            ps[:],
                            x_r[side][dk][:, lt * 128 : (lt + 1) * 128],
                            w[dk][:],
                            start=(dk == 0),
                            stop=(dk == DT - 1),
                        )
                    t = _tt(pers, [128, HPG, HD + 1], BF16, f"v{side}_{lt}")
                    nc.vector.tensor_copy(
                        t[:, :, 0:HD], ps[:].rearrange("p (h d) -> p h d", h=HPG)
                    )
                    nc.vector.tensor_copy(t[:, :, HD : HD + 1], onescol[:])
                    tiles.append(t)
                v_e[side] = tiles

            # ---- output accumulators (one [64, L] tile per head: keeps
            # base partition 0 so DVE tensor_tensor ops are legal) ----
            outacc = {
                qs: [_tt(pers, [HD, L], F32, f"out{qs}_{i}") for i in range(HPG)]
                for qs in (1, 2)
            }

            # ---- attention branches ----
            for h in range(HPG):
                po = (h % 2) * 64  # partition offset within the qkT tile pair
                ht = h // 2
                for ks in (1, 2):
                    kT = qkT[f"k{ks}"]
                    for qs in (1, 2):
                        qT = qkT[f"q{qs}"]
                        acc = _tt(accp, [HD + 1, L], F32, "acc")
                        for kt in range(KT):
                            st = _tt(stp, [128, L], F32, "st")
                            blk = h * KT + kt
                            lhsT = kz[ks][:, blk * 128 : (blk + 1) * 128]
                            for nh in range(2):
                                nc.tensor.matmul(
                                    st[:, nh * 512 : (nh + 1) * 512],
                                    lhsT,
                                    qT[ht][:, nh * 512 : (nh + 1) * 512],
                                    start=True,
                                    stop=True,
                                )
                            pt = _tt(ptp, [128, L], BF16, "pt")
                            nc.scalar.activation(
                                pt[:], st[:], EXP, bias=b_sb[ks][:, kt : kt + 1]
                            )
                            vt = v_e[ks][kt][:, h, :]
                            for nh in range(2):
                                nc.tensor.matmul(
                                    acc[:, nh * 512 : (nh + 1) * 512],
                                    vt,
                                    pt[:, nh * 512 : (nh + 1) * 512],
                                    start=(kt == 0),
                                    stop=(kt == KT - 1),
                                )
                        # normalization: r = 0.5 * mask_q / s, broadcast over
                        # partitions via a rank-1 PE matmul, then multiply.
                        # The reciprocal runs in a [128, 8] layout (DMA
                        # round-trip) — DVE RECIPROCAL cost scales with the
                        # free dim, so [1, 1024] would cost 6.5us.
                        s_sb = _tt(smp, [1, L], F32, "s_sb")
                        nc.vector.tensor_copy(s_sb[:], acc[HD : HD + 1, :])
                        s128 = _tt(smp, [128, KT], F32, "s128")
                        nc.sync.dma_start(s128[:], s_sb[:])
                        r128 = _tt(smp, [128, KT], F32, "r128")
                        nc.vector.reciprocal(r128[:], s128[:])
                        rsb = _tt(smp, [1, L], F32, "rsb")
                        nc.sync.dma_start(rsb[:], r128[:])
                        srm = _tt(smp, [1, L], F32R, "srm")
                        nc.vector.tensor_mul(srm[:], rsb[:], hm_sb[qs][:])
                        bc = _tt(stp, [64, L], F32, "st")
                        for nh in range(2):
                            nc.tensor.matmul(
                                bc[:, nh * 512 : (nh + 1) * 512],
                                ones64r[:],
                                srm[:, nh * 512 : (nh + 1) * 512],
                                start=True,
                                stop=True,
                            )
                        bc_sb = _tt(smp, [64, L], F32, "bc_sb")
                        nc.vector.tensor_copy(bc_sb[:], bc[:])
                        oslice = outacc[qs][h][:]
                        if ks == 1:
                            nc.vector.tensor_mul(oslice, acc[0:HD, :], bc_sb[:])
                        else:
                            tmp = _tt(smp, [64, L], F32, "tmp")
                            nc.vector.tensor_mul(tmp[:], acc[0:HD, :], bc_sb[:])
                            nc.vector.tensor_add(oslice, oslice, tmp[:])

            # ---- write outputs ----
            for qs, dram in ((1, out1T), (2, out2T)):
                for h in range(HPG):
                    nc.sync.dma_start(
                        dram[h * HD : (h + 1) * HD, :], outacc[qs][h][:]
                    )

    nc.compile()
    return nc


def kernel(**inputs):
    global _NC
    if _NC is None:
        _NC = _build()

    input1 = np.ascontiguousarray(inputs["input1"], dtype=np.float32)
    input2 = np.ascontiguousarray(inputs["input2"], dtype=np.float32)
    mask1 = np.asarray(inputs["mask1"], dtype=np.float32)
    mask2 = np.asarray(inputs["mask2"], dtype=np.float32)
    W = {k: np.asarray(inputs[k], dtype=np.float32) for k in
         ("Wq1", "Wk1", "Wv1", "Wq2", "Wk2", "Wv2")}

    in_maps = []
    for core in range(8):
        b, hg = core // 2, core % 2
        og = slice(hg * OG, (hg + 1) * OG)
        m = {
            "x1T": np.ascontiguousarray(input1[b].T),
            "x2T": np.ascontiguousarray(input2[b].T),
            "bias1": np.ascontiguousarray(
                ((mask1[b] - 1.0) * INF).reshape(KT, 128).T
            ),
            "bias2": np.ascontiguousarray(
                ((mask2[b] - 1.0) * INF).reshape(KT, 128).T
            ),
            "hm1": np.ascontiguousarray((0.5 * mask1[b]).reshape(1, L)),
            "hm2": np.ascontiguousarray((0.5 * mask2[b]).reshape(1, L)),
        }
        for wn in ("q1", "k1", "v1", "q2", "k2", "v2"):
            m["w" + wn] = np.ascontiguousarray(W["W" + wn[0] + wn[1]].T[:, og])
        in_maps.append(m)

    global LAST_RESULT
    if TRACE:
        _install_ntff_hook()
    res = run_bass_kernel_spmd(_NC, in_maps, list(range(8)), trace=TRACE)
    LAST_RESULT = res

    output1 = np.empty((NB, L, D), dtype=np.float32)
    output2 = np.empty((NB, L, D), dtype=np.float32)
    for core in range(8):
        b, hg = core // 2, core % 2
        og = slice(hg * OG, (hg + 1) * OG)
        output1[b, :, og] = res.results[core]["out1T"].T
        output2[b, :, og] = res.results[core]["out2T"].T
    return (output1, output2)



# revision 14
# speedup vs baseline: 2.0337x; 2.0337x over previous
# Trainium2 Bass kernel for the 4-branch cross-attention block.
#
# Problem: N=4 batches, L1=L2=1024, D=512, H=8 heads of 64.
#   q1,k1,v1 = proj(input1); q2,k2,v2 = proj(input2)
#   four attention branches (q1k1v1, q1k2v2, q2k1v1, q2k2v2), masked softmax
#   over the key axis, outputs averaged pairwise.
#
# Sharding: 8 cores = 4 batches x 2 head-groups (4 heads each). SPMD — one
# program, per-core data.
#
# Device-side dataflow (per core, 16 branch-heads of L x L attention):
#   ST   = K @ Q^T          (keys on partitions, queries on the free axis;
#                            lhsT = zero-padded kz block, rhs = qT, f32r)
#   P    = exp(ST)          (ACT engine; host pre-zeroed masked tokens in x,
#                            so masked keys give exp(0)=1 against v=0 rows
#                            and a masked ones-column — they drop out of both
#                            the numerator and the denominator exactly)
#   O^T  = [V | m]^T @ P    (bf16; mask column yields denominators in row 64)
#   r    = 0.5*mask_q * approx_recip(denom)        (DVE, no DMA round trips)
#   bc   = ones64^T @ r     (rank-1 PE broadcast of r over 64 partitions)
#   out += O^T * bc         (DVE, bf16 accumulator)
# The exp on ACT (128 tiles of [128,1024] @ ~1.3us) is the pipeline floor;
# QK/PV are emitted so the PE runs ahead of ACT and never blocks it.

import sys

sys.path.insert(0, "/opt/trn_rl_repo")

import numpy as np

import concourse.bacc as bacc
import concourse.mybir as mybir
import concourse.tile as tile
from concourse.bass_utils import run_bass_kernel_spmd

F32 = mybir.dt.float32
F32R = mybir.dt.float32r
BF16 = mybir.dt.bfloat16
EXP = mybir.ActivationFunctionType.Exp

L = 1024  # sequence length (both sides)
D = 512  # hidden
NB = 4  # batches
HPG = 4  # heads per core (head group)
HD = 64  # head size
OG = HPG * HD  # output channels per core = 256
KT = L // 128  # 8 key tiles
DT = D // 128  # 4 contraction tiles for projections
INF = 10000.0

_NC = None  # cached compiled program
TRACE = False  # set by test harness to capture an NTFF profile
LAST_RESULT = None  # full BassKernelResults of the last run (for profiling)
DEBUG_DUMP = False  # dump branch-0 intermediates to DRAM for debugging


def _tt(pool, shape, dtype, tag):
    return pool.tile(shape, dtype, tag=tag, name=tag)


def _install_ntff_hook():
    # antenv.axon_hooks is absent in this image; provide it so
    # run_bass_kernel_spmd(trace=True) can capture NTFF profiles.
    import types, contextlib, ctypes

    if "antenv.axon_hooks" in sys.modules:
        return
    lib = ctypes.CDLL("/opt/axon/libaxon_pjrt.so")
    lib.axon_start_nrt_profile.argtypes = [
        ctypes.POINTER(ctypes.c_int64),
        ctypes.c_size_t,
    ]
    lib.axon_start_nrt_profile.restype = ctypes.c_int64
    lib.axon_stop_nrt_profile.argtypes = [ctypes.c_char_p]
    lib.axon_stop_nrt_profile.restype = ctypes.c_int64

    @contextlib.contextmanager
    def _hook(output_dir, device_ids):
        import jax

        jax.devices()
        if device_ids:
            ids = (ctypes.c_int64 * len(device_ids))(*device_ids)
            rc = lib.axon_start_nrt_profile(ids, len(device_ids))
        else:
            rc = lib.axon_start_nrt_profile(None, 0)
        if rc != 0:
            raise RuntimeError(f"axon_start_nrt_profile rc={rc}")
        try:
            yield
        finally:
            n = lib.axon_stop_nrt_profile(str(output_dir).encode())
            print(f"ntff profile: {n} file(s) in {output_dir}", file=sys.stderr)

    mod = types.ModuleType("antenv.axon_hooks")
    mod.get_axon_ntff_profile_hook = lambda: _hook
    mod.set_axon_ntff_profile_hook = lambda h: None
    sys.modules["antenv.axon_hooks"] = mod


def _build():
    nc = bacc.Bacc("TRN2", target_bir_lowering=False, debug=False, num_devices=8)

    # f32r is bit-identical to f32 in SBUF/DRAM (PE rounds on load), so raw
    # f32 host data can be DMA'd straight into matmul operands — no casts.
    x_d = {s: nc.declare_dram_parameter(f"x{s}T", [D, L], F32R, isOutput=False)
           for s in (1, 2)}
    ws = {}
    for wn in ("wq1", "wk1", "wv1", "wq2", "wk2", "wv2"):
        ws[wn] = nc.declare_dram_parameter(wn, [D, OG], F32R, isOutput=False)
    hm_d = {s: nc.declare_dram_parameter(f"hm{s}", [1, L], F32, isOutput=False)
            for s in (1, 2)}
    m4_d = {s: nc.declare_dram_parameter(f"m4{s}", [128, KT * HPG], F32,
                                         isOutput=False) for s in (1, 2)}
    out_d = {s: nc.declare_dram_parameter(f"out{s}T", [OG, L], BF16, isOutput=True)
             for s in (1, 2)}
    dbg = {}
    if DEBUG_DUMP:
        for dn, shape, dt in (
            ("d_st", [128, L], F32), ("d_pt", [128, L], BF16),
            ("d_acc", [HD + 1, L], F32), ("d_rr", [1, L], F32),
            ("d_kz", [128, HPG * KT * 128], F32), ("d_q", [128, 2 * L], F32),
            ("d_ve", [128, HPG * (HD + 1)], BF16), ("d_rr2", [1, L], F32),
        ):
            dbg[dn] = nc.declare_dram_parameter(dn, shape, dt, isOutput=True)

    with tile.TileContext(nc) as tc:
        with (
            tc.tile_pool(name="pers", bufs=1) as pers,
            tc.tile_pool(name="pt", bufs=4) as ptp,
            tc.tile_pool(name="sm", bufs=2) as smp,
            tc.tile_pool(name="st", bufs=2, space="PSUM") as stp,
            tc.tile_pool(name="acc", bufs=2, space="PSUM") as accp,
        ):
            # ---- input DMAs (f32 bits land directly in f32r tiles) ----
            x_r = {}
            for s in (1, 2):
                t = _tt(pers, [128, DT, L], F32R, f"x{s}")
                for dk in range(DT):
                    nc.sync.dma_start(t[:, dk, :], x_d[s][dk * 128:(dk + 1) * 128, :])
                x_r[s] = t
            w_r = {}
            for wn in ws:
                t = _tt(pers, [128, DT, OG], F32R, wn)
                for dk in range(DT):
                    nc.sync.dma_start(t[:, dk, :], ws[wn][dk * 128:(dk + 1) * 128, :])
                w_r[wn] = t
            hm_sb = {}
            for s in (1, 2):
                t = _tt(pers, [1, L], F32, f"hm{s}")
                nc.sync.dma_start(t[:], hm_d[s][:])
                hm_sb[s] = t
            m4_sb = {}
            for s in (1, 2):
                t = _tt(pers, [128, KT, HPG], F32, f"m4{s}")
                nc.sync.dma_start(t[:].rearrange("p a b -> p (a b)"), m4_d[s][:])
                m4_sb[s] = t

            # ---- small constants ----
            ones64f = _tt(pers, [1, 64], F32, "ones64f")
            nc.vector.memset(ones64f[:], 1.0)
            ones64r = _tt(pers, [1, 64], F32R, "ones64r")
            nc.vector.tensor_copy(ones64r[:], ones64f[:])

            # ---- projections ----
            # qT per side: [128, 2, L] (tile ht holds heads 2ht, 2ht+1).
            # kz per side: [128, HPG*KT*128] zero-padded per (head, kt) block
            # so QK's moving qT streams all 128 partitions at full rate.
            kz = {}
            for s in (1, 2):
                z = _tt(pers, [128, HPG * KT * 128], F32R, f"kz{s}")
                nc.gpsimd.memset(z[:].bitcast(F32), 0.0)
                kz[s] = z
            qT = {}
            for s in (1, 2):
                qt = _tt(pers, [128, 2, L], F32R, f"q{s}T")
                for ot in range(2):
                    for name, wkey in (("q", f"wq{s}"), ("k", f"wk{s}")):
                        w = w_r[wkey]
                        ps = _tt(stp, [128, L], F32, "st")
                        for dk in range(DT):
                            for nh in range(2):
                                nc.tensor.matmul(
                                    ps[:, nh * 512:(nh + 1) * 512],
                                    w[:, dk, ot * 128:(ot + 1) * 128],
                                    x_r[s][:, dk, nh * 512:(nh + 1) * 512],
                                    start=(dk == 0),
                                    stop=(dk == DT - 1),
                                )
                        if name == "q":
                            nc.vector.tensor_copy(qt[:, ot, :], ps[:])
                        else:
                            # head 2ot -> partitions 0:64, block col (2ot)*KT*128
                            # head 2ot+1 -> partitions 64:128, next block
                            for hh in range(2):
                                h = 2 * ot + hh
                                po = hh * 64
                                nc.vector.tensor_copy(
                                    kz[s][po:po + 64, h * KT * 128:(h + 1) * KT * 128],
                                    ps[po:po + 64, :],
                                )
                qT[s] = qt

            # v in natural layout with mask column: [128, HPG, 65] per key tile
            v_e = {}
            for s in (1, 2):
                w = w_r[f"wv{s}"]
                tiles = []
                for lt in range(KT):
                    ps = _tt(stp, [128, OG], F32, "st")
                    for dk in range(DT):
                        nc.tensor.matmul(
                            ps[:],
                            x_r[s][:, dk, lt * 128:(lt + 1) * 128],
                            w[:, dk, :],
                            start=(dk == 0),
                            stop=(dk == DT - 1),
                        )
                    t = _tt(pers, [128, HPG, HD + 1], BF16, f"v{s}_{lt}")
                    nc.vector.tensor_copy(
                        t[:, :, 0:HD], ps[:].rearrange("p (h d) -> p h d", h=HPG)
                    )
                    nc.vector.tensor_copy(t[:, :, HD:HD + 1],
                                          m4_sb[s][:, lt, :, None])
                    tiles.append(t)
                v_e[s] = tiles

            # ---- output accumulators (bf16; [64, L] per (qs, head)) ----
            outacc = {
                qs: [_tt(pers, [HD, L], BF16, f"out{qs}_{i}") for i in range(HPG)]
                for qs in (1, 2)
            }

            # ---- attention branches ----
            branches = [(h, ks, qs) for h in range(HPG) for ks in (1, 2)
                        for qs in (1, 2)]
            pend = None  # deferred normalize state of the previous branch

            def emit_bc_and_combine(p):
                # emitted one iteration late so the bc matmul (which waits on
                # DVE) never stalls the PE queue ahead of fresh QK work.
                h, ks, qs, acc, rr = p
                bc = _tt(stp, [64, L], F32, "st")
                for nh in range(2):
                    nc.tensor.matmul(
                        bc[:, nh * 512:(nh + 1) * 512],
                        ones64r[:],
                        rr[:, nh * 512:(nh + 1) * 512],
                        start=True,
                        stop=True,
                    )
                bc_sb = _tt(smp, [64, L], F32, "bc_sb")
                nc.vector.tensor_copy(bc_sb[:], bc[:])
                oslice = outacc[qs][h][:]
                if ks == 1:
                    nc.vector.tensor_mul(oslice, acc[0:HD, :], bc_sb[:])
                else:
                    tmp = _tt(smp, [64, L], BF16, "tmp")
                    nc.vector.tensor_mul(tmp[:], acc[0:HD, :], bc_sb[:])
                    nc.vector.tensor_add(oslice, oslice, tmp[:])
                    nc.sync.dma_start(out_d[qs][h * HD:(h + 1) * HD, :], oslice)

            def dump(dn, src, bounce=False):
                if dn not in dbg:
                    return
                if bounce:  # PSUM source: copy to SBUF first
                    t = _tt(pers, dbg[dn].shape, F32, dn)
                    nc.vector.tensor_copy(t[:], src)
                    src = t[:]
                nc.sync.dma_start(dbg[dn][:], src)

            if dbg:
                dump("d_kz", kz[1][:].bitcast(F32))
                dump("d_q", qT[1][:].rearrange("p a b -> p (a b)").bitcast(F32))
                dump("d_ve", v_e[1][0][:].rearrange("p a b -> p (a b)"))

            for bi, (h, ks, qs) in enumerate(branches):
                po = (h % 2) * 64
                ht = h // 2
                # QK for all 8 key tiles first: the PE free-runs ahead of ACT
                # (throttled only by the two st PSUM slots).
                sts = []
                for kt in range(KT):
                    st = _tt(stp, [128, L], F32, "st")
                    blk = (h * KT + kt) * 128
                    for nh in range(2):
                        nc.tensor.matmul(
                            st[:, nh * 512:(nh + 1) * 512],
                            kz[ks][:, blk:blk + 128],
                            qT[qs][:, ht, nh * 512:(nh + 1) * 512],
                            start=True,
                            stop=True,
                        )
                    pt = _tt(ptp, [128, L], BF16, "pt")
                    if bi == 0 and kt == 0:
                        dump("d_st", st[:], bounce=True)
                    nc.scalar.activation(pt[:], st[:], EXP)
                    if bi == 0 and kt == 0:
                        dump("d_pt", pt[:])
                    sts.append(pt)
                if pend is not None:
                    emit_bc_and_combine(pend)
                acc = _tt(accp, [HD + 1, L], F32, "acc")
                for kt in range(KT):
                    for nh in range(2):
                        nc.tensor.matmul(
                            acc[:, nh * 512:(nh + 1) * 512],
                            v_e[ks][kt][:, h, :],
                            sts[kt][:, nh * 512:(nh + 1) * 512],
                            start=(kt == 0),
                            stop=(kt == KT - 1),
                        )
                if bi == 0:
                    dump("d_acc", acc[:], bounce=True)
                # normalization scalars: r = 0.5 * mask_q / denom, in [1, L].
                s_sb = _tt(smp, [1, L], F32, "s_sb")
                nc.vector.tensor_copy(s_sb[:], acc[HD:HD + 1, :])
                rinv = _tt(smp, [1, L], F32, "rinv")
                nc.vector.reciprocal_approx_fast(rinv[:], s_sb[:])
                rr = _tt(smp, [1, L], F32R, "rr")
                nc.vector.tensor_mul(rr[:], rinv[:], hm_sb[qs][:])
                if bi == 0:
                    dump("d_rr", rr[:].bitcast(F32))
                    if "d_rr2" in dbg:
                        r2 = _tt(pers, [1, L], F32, "r2")
                        nc.vector.reciprocal(r2[:], acc[HD:HD + 1, :])
                        nc.sync.dma_start(dbg["d_rr2"][:], r2[:])
                pend = (h, ks, qs, acc, rr)
            emit_bc_and_combine(pend)

    nc.compile()
    return nc


def kernel(**inputs):
    global _NC
    if _NC is None:
        _NC = _build()

    mask1 = np.asarray(inputs["mask1"], dtype=np.float32)
    mask2 = np.asarray(inputs["mask2"], dtype=np.float32)
    # pre-zero masked tokens: masked keys then contribute exp(0)*0 = 0 to
    # both the attention numerator and (via the v mask column) denominator.
    x1 = np.asarray(inputs["input1"], dtype=np.float32) * mask1[:, :, None]
    x2 = np.asarray(inputs["input2"], dtype=np.float32) * mask2[:, :, None]
    W = {k: np.asarray(inputs[k], dtype=np.float32) for k in
         ("Wq1", "Wk1", "Wv1", "Wq2", "Wk2", "Wv2")}

    in_maps = []
    for core in range(8):
        b, hg = core // 2, core % 2
        og = slice(hg * OG, (hg + 1) * OG)
        m = {
            "x1T": np.ascontiguousarray(x1[b].T),
            "x2T": np.ascontiguousarray(x2[b].T),
            "hm1": np.ascontiguousarray((0.5 * mask1[b]).reshape(1, L)),
            "hm2": np.ascontiguousarray((0.5 * mask2[b]).reshape(1, L)),
            "m41": np.ascontiguousarray(
                np.repeat(mask1[b].reshape(KT, 128).T[:, :, None], HPG, axis=2)
                .reshape(128, KT * HPG)),
            "m42": np.ascontiguousarray(
                np.repeat(mask2[b].reshape(KT, 128).T[:, :, None], HPG, axis=2)
                .reshape(128, KT * HPG)),
        }
        for wn in ("q1", "k1", "v1", "q2", "k2", "v2"):
            m["w" + wn] = np.ascontiguousarray(W["W" + wn[0] + wn[1]].T[:, og])
        in_maps.append(m)

    global LAST_RESULT
    if TRACE:
        _install_ntff_hook()
    res = run_bass_kernel_spmd(_NC, in_maps, list(range(8)), trace=TRACE)
    LAST_RESULT = res

    output1 = np.empty((NB, L, D), dtype=np.float32)
    output2 = np.empty((NB, L, D), dtype=np.float32)
    for core in range(8):
        b, hg = core // 2, core % 2
        og = slice(hg * OG, (hg + 1) * OG)
        output1[b, :, og] = np.asarray(res.results[core]["out1T"],
                                       dtype=np.float32).T
        output2[b, :, og] = np.asarray(res.results[core]["out2T"],
                                       dtype=np.float32).T
    return (output1, output2)


# revision 15
# speedup vs baseline: 2.0861x; 1.0258x over previous
# Trainium2 Bass kernel for the 4-branch cross-attention block.
#
# Problem: N=4 batches, L1=L2=1024, D=512, H=8 heads of 64.
#   q1,k1,v1 = proj(input1); q2,k2,v2 = proj(input2)
#   four attention branches (q1k1v1, q1k2v2, q2k1v1, q2k2v2), masked softmax
#   over the key axis, outputs averaged pairwise.
#
# Sharding: 8 cores = 4 batches x 2 head-groups (4 heads each). SPMD — one
# program, per-core data.
#
# Device-side dataflow (per core, 16 branch-heads of L x L attention):
#   ST   = K @ Q^T          (keys on partitions, queries on the free axis;
#                            lhsT = zero-padded kz block, rhs = qT, f32r)
#   P    = exp(ST)          (ACT engine; host pre-zeroed masked tokens in x,
#                            so masked keys give exp(0)=1 against v=0 rows
#                            and a masked ones-column — they drop out of both
#                            the numerator and the denominator exactly)
#   O^T  = [V | m]^T @ P    (bf16; mask column yields denominators in row 64)
#   r    = 0.5*mask_q * approx_recip(denom)        (DVE, no DMA round trips)
#   bc   = ones64^T @ r     (rank-1 PE broadcast of r over 64 partitions)
#   out += O^T * bc         (DVE, bf16 accumulator)
# The exp on ACT (128 tiles of [128,1024] @ ~1.3us) is the pipeline floor;
# QK/PV are emitted so the PE runs ahead of ACT and never blocks it.

import sys

sys.path.insert(0, "/opt/trn_rl_repo")

import numpy as np

import concourse.bacc as bacc
import concourse.mybir as mybir
import concourse.tile as tile
from concourse.bass_utils import run_bass_kernel_spmd

F32 = mybir.dt.float32
F32R = mybir.dt.float32r
F16 = mybir.dt.float16
BF16 = mybir.dt.bfloat16
EXP = mybir.ActivationFunctionType.Exp

L = 1024  # sequence length (both sides)
D = 512  # hidden
NB = 4  # batches
HPG = 4  # heads per core (head group)
HD = 64  # head size
OG = HPG * HD  # output channels per core = 256
KT = L // 128  # 8 key tiles
DT = D // 128  # 4 contraction tiles for projections
INF = 10000.0

_NC = None  # cached compiled program
TRACE = False  # set by test harness to capture an NTFF profile
LAST_RESULT = None  # full BassKernelResults of the last run (for profiling)
DEBUG_DUMP = False  # dump branch-0 intermediates to DRAM for debugging


def _tt(pool, shape, dtype, tag):
    return pool.tile(shape, dtype, tag=tag, name=tag)


def _install_ntff_hook():
    # antenv.axon_hooks is absent in this image; provide it so
    # run_bass_kernel_spmd(trace=True) can capture NTFF profiles.
    import types, contextlib, ctypes

    if "antenv.axon_hooks" in sys.modules:
        return
    lib = ctypes.CDLL("/opt/axon/libaxon_pjrt.so")
    lib.axon_start_nrt_profile.argtypes = [
        ctypes.POINTER(ctypes.c_int64),
        ctypes.c_size_t,
    ]
    lib.axon_start_nrt_profile.restype = ctypes.c_int64
    lib.axon_stop_nrt_profile.argtypes = [ctypes.c_char_p]
    lib.axon_stop_nrt_profile.restype = ctypes.c_int64

    @contextlib.contextmanager
    def _hook(output_dir, device_ids):
        import jax

        jax.devices()
        if device_ids:
            ids = (ctypes.c_int64 * len(device_ids))(*device_ids)
            rc = lib.axon_start_nrt_profile(ids, len(device_ids))
        else:
            rc = lib.axon_start_nrt_profile(None, 0)
        if rc != 0:
            raise RuntimeError(f"axon_start_nrt_profile rc={rc}")
        try:
            yield
        finally:
            n = lib.axon_stop_nrt_profile(str(output_dir).encode())
            print(f"ntff profile: {n} file(s) in {output_dir}", file=sys.stderr)

    mod = types.ModuleType("antenv.axon_hooks")
    mod.get_axon_ntff_profile_hook = lambda: _hook
    mod.set_axon_ntff_profile_hook = lambda h: None
    sys.modules["antenv.axon_hooks"] = mod


def _build():
    nc = bacc.Bacc("TRN2", target_bir_lowering=False, debug=False, num_devices=8)

    # f32r is bit-identical to f32 in SBUF/DRAM (PE rounds on load), so raw
    # f32 host data can be DMA'd straight into matmul operands — no casts.
    x_d = {s: nc.declare_dram_parameter(f"x{s}T", [D, L], F32R, isOutput=False)
           for s in (1, 2)}
    ws = {}
    for wn in ("wq1", "wk1", "wv1", "wq2", "wk2", "wv2"):
        ws[wn] = nc.declare_dram_parameter(wn, [D, OG], F32R, isOutput=False)
    hm_d = {s: nc.declare_dram_parameter(f"hm{s}", [1, L], F32, isOutput=False)
            for s in (1, 2)}
    m4_d = {s: nc.declare_dram_parameter(f"m4{s}", [128, KT * HPG], F32,
                                         isOutput=False) for s in (1, 2)}
    out_d = {s: nc.declare_dram_parameter(f"out{s}T", [OG, L], BF16, isOutput=True)
             for s in (1, 2)}
    dbg = {}
    if DEBUG_DUMP:
        for dn, shape, dt in (
            ("d_st", [128, L], F32), ("d_pt", [128, L], BF16),
            ("d_acc", [HD + 1, L], F32), ("d_rr", [1, L], F32),
            ("d_kz", [128, HPG * KT * 128], F32), ("d_q", [128, 2 * L], F32),
            ("d_ve", [128, HPG * (HD + 1)], BF16), ("d_rr2", [1, L], F32),
        ):
            dbg[dn] = nc.declare_dram_parameter(dn, shape, dt, isOutput=True)

    with tile.TileContext(nc) as tc:
        with (
            tc.tile_pool(name="pers", bufs=1) as pers,
            tc.tile_pool(name="pt", bufs=4) as ptp,
            tc.tile_pool(name="sm", bufs=2) as smp,
            tc.tile_pool(name="st", bufs=2, space="PSUM") as stp,
            tc.tile_pool(name="acc", bufs=2, space="PSUM") as accp,
        ):
            # ---- input DMAs (f32 bits land directly in f32r tiles) ----
            x_r = {}
            for s in (1, 2):
                t = _tt(pers, [128, DT, L], F32R, f"x{s}")
                for dk in range(DT):
                    nc.sync.dma_start(t[:, dk, :], x_d[s][dk * 128:(dk + 1) * 128, :])
                x_r[s] = t
            w_r = {}
            for wn in ws:
                t = _tt(pers, [128, DT, OG], F32R, wn)
                for dk in range(DT):
                    nc.sync.dma_start(t[:, dk, :], ws[wn][dk * 128:(dk + 1) * 128, :])
                w_r[wn] = t
            hm_sb = {}
            for s in (1, 2):
                t = _tt(pers, [1, L], F32, f"hm{s}")
                nc.sync.dma_start(t[:], hm_d[s][:])
                hm_sb[s] = t
            m4_sb = {}
            for s in (1, 2):
                t = _tt(pers, [128, KT, HPG], F32, f"m4{s}")
                nc.sync.dma_start(t[:].rearrange("p a b -> p (a b)"), m4_d[s][:])
                m4_sb[s] = t

            # ---- small constants ----
            ones64f = _tt(pers, [1, 64], F32, "ones64f")
            nc.vector.memset(ones64f[:], 1.0)
            ones64r = _tt(pers, [1, 64], F32R, "ones64r")
            nc.vector.tensor_copy(ones64r[:], ones64f[:])

            # ---- projections ----
            # qT per side: [128, 2, L] (tile ht holds heads 2ht, 2ht+1).
            # kz per side: [128, HPG*KT*128] zero-padded per (head, kt) block
            # so QK's moving qT streams all 128 partitions at full rate.
            kz = {}
            for s in (1, 2):
                z = _tt(pers, [128, HPG * KT * 128], F16, f"kz{s}")
                nc.gpsimd.memset(z[:], 0.0)
                kz[s] = z
            qT = {}
            for s in (1, 2):
                qt = _tt(pers, [128, 2, L], F16, f"q{s}T")
                for ot in range(2):
                    for name, wkey in (("q", f"wq{s}"), ("k", f"wk{s}")):
                        w = w_r[wkey]
                        ps = _tt(stp, [128, L], F32, "st")
                        for dk in range(DT):
                            for nh in range(2):
                                nc.tensor.matmul(
                                    ps[:, nh * 512:(nh + 1) * 512],
                                    w[:, dk, ot * 128:(ot + 1) * 128],
                                    x_r[s][:, dk, nh * 512:(nh + 1) * 512],
                                    start=(dk == 0),
                                    stop=(dk == DT - 1),
                                )
                        if name == "q":
                            nc.vector.tensor_copy(qt[:, ot, :], ps[:])
                        else:
                            # head 2ot -> partitions 0:64, block col (2ot)*KT*128
                            # head 2ot+1 -> partitions 64:128, next block
                            for hh in range(2):
                                h = 2 * ot + hh
                                po = hh * 64
                                nc.vector.tensor_copy(
                                    kz[s][po:po + 64, h * KT * 128:(h + 1) * KT * 128],
                                    ps[po:po + 64, :],
                                )
                qT[s] = qt

            # v in natural layout with mask column: [128, HPG, 65] per key tile
            v_e = {}
            for s in (1, 2):
                w = w_r[f"wv{s}"]
                tiles = []
                for lt in range(KT):
                    ps = _tt(stp, [128, OG], F32, "st")
                    for dk in range(DT):
                        nc.tensor.matmul(
                            ps[:],
                            x_r[s][:, dk, lt * 128:(lt + 1) * 128],
                            w[:, dk, :],
                            start=(dk == 0),
                            stop=(dk == DT - 1),
                        )
                    t = _tt(pers, [128, HPG, HD + 1], BF16, f"v{s}_{lt}")
                    nc.vector.tensor_copy(
                        t[:, :, 0:HD], ps[:].rearrange("p (h d) -> p h d", h=HPG)
                    )
                    nc.vector.tensor_copy(t[:, :, HD:HD + 1],
                                          m4_sb[s][:, lt, :, None])
                    tiles.append(t)
                v_e[s] = tiles

            # ---- output accumulators (bf16; [64, L] per (qs, head)) ----
            outacc = {
                qs: [_tt(pers, [HD, L], BF16, f"out{qs}_{i}") for i in range(HPG)]
                for qs in (1, 2)
            }

            # ---- attention branches ----
            branches = [(h, ks, qs) for h in range(HPG) for ks in (1, 2)
                        for qs in (1, 2)]
            pend = None  # deferred normalize state of the previous branch

            def emit_bc_and_combine(p):
                # emitted one iteration late so the bc matmul (which waits on
                # DVE) never stalls the PE queue ahead of fresh QK work.
                h, ks, qs, acc, rr = p
                bc = _tt(stp, [64, L], F32, "st")
                for nh in range(2):
                    nc.tensor.matmul(
                        bc[:, nh * 512:(nh + 1) * 512],
                        ones64r[:],
                        rr[:, nh * 512:(nh + 1) * 512],
                        start=True,
                        stop=True,
                    )
                bc_sb = _tt(smp, [64, L], F32, "bc_sb")
                nc.vector.tensor_copy(bc_sb[:], bc[:])
                oslice = outacc[qs][h][:]
                if ks == 1:
                    nc.vector.tensor_mul(oslice, acc[0:HD, :], bc_sb[:])
                else:
                    tmp = _tt(smp, [64, L], BF16, "tmp")
                    nc.vector.tensor_mul(tmp[:], acc[0:HD, :], bc_sb[:])
                    nc.vector.tensor_add(oslice, oslice, tmp[:])
                    nc.sync.dma_start(out_d[qs][h * HD:(h + 1) * HD, :], oslice)

            def dump(dn, src, bounce=False):
                if dn not in dbg:
                    return
                if bounce:  # PSUM source: copy to SBUF first
                    t = _tt(pers, dbg[dn].shape, F32, dn)
                    nc.vector.tensor_copy(t[:], src)
                    src = t[:]
                nc.sync.dma_start(dbg[dn][:], src)

            if dbg:
                dump("d_kz", kz[1][:], bounce=True)
                dump("d_q", qT[1][:].rearrange("p a b -> p (a b)"), bounce=True)
                dump("d_ve", v_e[1][0][:].rearrange("p a b -> p (a b)"))

            for bi, (h, ks, qs) in enumerate(branches):
                po = (h % 2) * 64
                ht = h // 2
                # QK for all 8 key tiles first: the PE free-runs ahead of ACT
                # (throttled only by the two st PSUM slots).
                sts = []
                for kt in range(KT):
                    st = _tt(stp, [128, L], F32, "st")
                    blk = (h * KT + kt) * 128
                    for nh in range(2):
                        nc.tensor.matmul(
                            st[:, nh * 512:(nh + 1) * 512],
                            kz[ks][:, blk:blk + 128],
                            qT[qs][:, ht, nh * 512:(nh + 1) * 512],
                            start=True,
                            stop=True,
                        )
                    pt = _tt(ptp, [128, L], BF16, "pt")
                    if bi == 0 and kt == 0:
                        dump("d_st", st[:], bounce=True)
                    nc.scalar.activation(pt[:], st[:], EXP)
                    if bi == 0 and kt == 0:
                        dump("d_pt", pt[:])
                    sts.append(pt)
                if pend is not None:
                    emit_bc_and_combine(pend)
                acc = _tt(accp, [HD + 1, L], F32, "acc")
                for kt in range(KT):
                    for nh in range(2):
                        nc.tensor.matmul(
                            acc[:, nh * 512:(nh + 1) * 512],
                            v_e[ks][kt][:, h, :],
                            sts[kt][:, nh * 512:(nh + 1) * 512],
                            start=(kt == 0),
                            stop=(kt == KT - 1),
                        )
                if bi == 0:
                    dump("d_acc", acc[:], bounce=True)
                # normalization scalars: r = 0.5 * mask_q / denom, in [1, L].
                s_sb = _tt(smp, [1, L], F32, "s_sb")
                nc.vector.tensor_copy(s_sb[:], acc[HD:HD + 1, :])
                rinv = _tt(smp, [1, L], F32, "rinv")
                nc.vector.reciprocal_approx_fast(rinv[:], s_sb[:])
                rr = _tt(smp, [1, L], F32R, "rr")
                nc.vector.tensor_mul(rr[:], rinv[:], hm_sb[qs][:])
                if bi == 0:
                    dump("d_rr", rr[:].bitcast(F32))
                    if "d_rr2" in dbg:
                        r2 = _tt(pers, [1, L], F32, "r2")
                        nc.vector.reciprocal(r2[:], acc[HD:HD + 1, :])
                        nc.sync.dma_start(dbg["d_rr2"][:], r2[:])
                pend = (h, ks, qs, acc, rr)
            emit_bc_and_combine(pend)

    nc.compile()
    return nc


def kernel(**inputs):
    global _NC
    if _NC is None:
        _NC = _build()

    mask1 = np.asarray(inputs["mask1"], dtype=np.float32)
    mask2 = np.asarray(inputs["mask2"], dtype=np.float32)
    # pre-zero masked tokens: masked keys then contribute exp(0)*0 = 0 to
    # both the attention numerator and (via the v mask column) denominator.
    x1 = np.asarray(inputs["input1"], dtype=np.float32) * mask1[:, :, None]
    x2 = np.asarray(inputs["input2"], dtype=np.float32) * mask2[:, :, None]
    W = {k: np.asarray(inputs[k], dtype=np.float32) for k in
         ("Wq1", "Wk1", "Wv1", "Wq2", "Wk2", "Wv2")}

    in_maps = []
    for core in range(8):
        b, hg = core // 2, core % 2
        og = slice(hg * OG, (hg + 1) * OG)
        m = {
            "x1T": np.ascontiguousarray(x1[b].T),
            "x2T": np.ascontiguousarray(x2[b].T),
            "hm1": np.ascontiguousarray((0.5 * mask1[b]).reshape(1, L)),
            "hm2": np.ascontiguousarray((0.5 * mask2[b]).reshape(1, L)),
            "m41": np.ascontiguousarray(
                np.repeat(mask1[b].reshape(KT, 128).T[:, :, None], HPG, axis=2)
                .reshape(128, KT * HPG)),
            "m42": np.ascontiguousarray(
                np.repeat(mask2[b].reshape(KT, 128).T[:, :, None], HPG, axis=2)
                .reshape(128, KT * HPG)),
        }
        for wn in ("q1", "k1", "v1", "q2", "k2", "v2"):
            m["w" + wn] = np.ascontiguousarray(W["W" + wn[0] + wn[1]].T[:, og])
        in_maps.append(m)

    global LAST_RESULT
    if TRACE:
        _install_ntff_hook()
    res = run_bass_kernel_spmd(_NC, in_maps, list(range(8)), trace=TRACE)
    LAST_RESULT = res

    output1 = np.empty((NB, L, D), dtype=np.float32)
    output2 = np.empty((NB, L, D), dtype=np.float32)
    for core in range(8):
        b, hg = core // 2, core % 2
        og = slice(hg * OG, (hg + 1) * OG)
        output1[b, :, og] = np.asarray(res.results[core]["out1T"],
                                       dtype=np.float32).T
        output2[b, :, og] = np.asarray(res.results[core]["out2T"],
                                       dtype=np.float32).T
    return (output1, output2)
